# revision 13
# baseline (speedup 1.0000x reference)
"""Trainium2 Bass kernel for nn_DualDescriptorTS.

Math:  Nk[b,i] = sum_{j,g} x[b,j] * P[i,j,g] * cos(2*pi*k[b]/p[i,j,g]),
       p[i,j,g] = i*1024 + j*16 + g + 2,  x = emb[token_indices].

Factorization (k = arange(B), b = 32*h + l): angle addition gives each
(i, j) slab D_{i,j}[l, h] = stat^T mov with mov P-independent and
numerically low rank (exactly 2 at a 1% tail for every i >= 2 slab;
only i in {0, 1} are heavier).  mov ~ A @ Q via SVD; P folds into the
tiny stationary factor statp = A^T stat per call.

v5 design (trace-driven):
  * Merged matmuls: the 4 ccol slabs of a (rg, half, sh) slot run as
    ONE matmul with a block-diagonal stationary [Ktot<=38, 128];
    heavy chains (i=0,1) spread one-per-core.  PE sustains ~107ns
    per LDWEIGHTS+MATMUL pair -> ~1.7us/rg.
  * One [128, 2048] PSUM tile per rg (4 banks, bufs=2): ONE scalar
    ACT copy downconverts the rg to fp16 (~1.97us), vector does ONE
    fp16 2x-mode multiply by x (~1.2us) and the whole fold
    (t1 [1024] + t2 [512], ~1.03us).  GpSimd is kept off the
    elementwise path entirely: measured SBUF contention bloats
    concurrent vector ops ~3x, costing more than gpsimd contributes.
  * Chains of 4 slabs {j, j+16, j+32, j+48}: the on-device fold
    cuts output to 128KB/rg.  The DMA fabric measures ~110GB/s
    aggregate with 2-4KB row-packets, so 2MB of output (chains-2)
    was the binding tail; 1MB clears it.  Host sums 16 partials/row.
  * Input DMA: one ring (sync), strict need order, sized so each
    table lands just before its row-group's matmuls; rg0 (heavy
    tables) processed late.  xa rides the same ring in two halves.
  * ~8us of framework preamble/epilogue (257 sem clears) is fixed.
"""
import numpy as np
import ml_dtypes

import concourse.bacc as bacc
import concourse.tile as tile
from concourse import mybir
from concourse.bass_utils import run_bass_kernel_spmd

F32 = mybir.dt.float32
BF16 = mybir.dt.bfloat16
FP16 = mybir.dt.float16
TWO_PI = 2.0 * np.pi

M, O, B = 64, 16, 4096
NC = 8            # cores
NI = 8            # row-groups (rgs) per core
NH, NL = 128, 32  # b = 32*h + l
TAU = 0.01        # relative Frobenius tail kept when truncating mov
GRP = [2, 4, 10]  # rg0 mms per depth-sorted rect (ha, hb1, hb2)

_bf16 = ml_dtypes.bfloat16
_fp16 = np.float16
_cache = {}
_last_results = None

RGORDER = [1, 2, 3, 4, 5, 0, 6, 7]


def _heavy_core(i, sg2, ccol):
    if i == 0:
        return ccol + 4 * (sg2 % 2)
    return ccol + 4 * ((sg2 + 1) % 2)


def _shapes(HA, HB1, HB2):
    return {"l1a": (8, 2048), "l1b": (8, 2048), "l2a": (8, 4096),
            "l2b": (8, 4096), "l3a": (8, 8192), "l3b": (8, 8192),
            "ha": (HA, GRP[0] * 256), "hb1": (HB1, GRP[1] * 256),
            "hb2": (HB2, GRP[2] * 256)}


def _layout(rg, half, sh, pos0):
    """(tensor_name, mov_col, stat_col) for mm (rg, half, sh)."""
    if rg == 0:
        n = int(pos0[8 * half + sh])
        if n < GRP[0]:
            return "ha", 256 * n, 256 * n + 128
        n -= GRP[0]
        if n < GRP[1]:
            return "hb1", 256 * n, 256 * n + 128
        n -= GRP[1]
        return "hb2", 256 * n, 256 * n + 128
    if rg == 1:
        name = "l1a" if half == 0 else "l1b"
        return name, 128 * sh, 1024 + 128 * sh
    if rg == 2:
        name, off = "l2a", 0
    elif rg == 3:
        name, off = "l2b", 0
    elif rg <= 5:
        name, off = "l3a", 4096 * (rg - 4)
    else:
        name, off = "l3b", 4096 * (rg - 6)
    mc = off + 2048 * half + 128 * sh
    return name, mc, mc + 1024


def _factors():
    if "fac" in _cache:
        return _cache["fac"]
    h = np.arange(NH, dtype=np.float64)
    l = np.arange(NL, dtype=np.float64)
    ig = np.arange(M, dtype=np.float64)[:, None, None]
    jg = np.arange(M, dtype=np.float64)[None, :, None]
    gg = np.arange(O, dtype=np.float64)[None, None, :]
    theta = TWO_PI / (1024.0 * ig + 16.0 * jg + gg + 2.0)  # [i, j, g]
    a1 = theta[..., None] * (32.0 * h)
    mov = np.concatenate([np.cos(a1), np.sin(a1)], axis=2).reshape(
        M * M, 32, NH)
    U, S, Vt = np.linalg.svd(mov, full_matrices=False)
    fro = np.sqrt((S ** 2).sum(1))
    tail = np.sqrt(np.cumsum((S ** 2)[:, ::-1], axis=1))[:, ::-1] / fro[:, None]
    Ks = np.maximum(
        np.array([np.searchsorted(-tail[s], -TAU) for s in range(M * M)]), 1)
    Ksq = Ks.reshape(M, M)
    A = (U * S[:, None, :])                                 # [4096, 32, 32]

    thl = theta.reshape(M * M, O)[:, :, None] * l           # [s, g, l]
    cl, sl = np.cos(thl), np.sin(thl)
    G01 = (A[:128, :O, :, None] * cl[:128, :, None, :]
           - A[:128, O:, :, None] * sl[:128, :, None, :]).astype(np.float32)
    G2 = (A[128:, :O, :2, None] * cl[128:, :, None, :]
          - A[128:, O:, :2, None] * sl[128:, :, None, :]).astype(np.float32)

    # chain map: imap[c, rg, sg2, ccol] = i; chain covers the 4 slabs
    # j = 4*(8*half + sg2 + 4*m') + ccol, i.e. {j0, j0+16, j0+32, j0+48}
    imap = np.zeros((NC, NI, 4, 4), dtype=np.int64)
    for sg2 in range(4):
        for ccol in range(4):
            imap[_heavy_core(0, sg2, ccol), 0, sg2, ccol] = 0
            imap[_heavy_core(1, sg2, ccol), 0, sg2, ccol] = 1
            light = []
            for rg in range(NI):
                for c in range(NC):
                    if rg == 0 and (c == _heavy_core(0, sg2, ccol)
                                    or c == _heavy_core(1, sg2, ccol)):
                        continue
                    light.append((c, rg))
            assert len(light) == 62
            for n, (c, rg) in enumerate(light):
                imap[c, rg, sg2, ccol] = n + 2

    DEP = np.zeros((NI, 2, 8), dtype=np.int64)
    for rg in range(NI):
        for half in range(2):
            for sh in range(8):
                d = 0
                for c in range(NC):
                    dc = sum(int(Ksq[imap[c, rg, sh % 4, ccol],
                                     4 * (8 * half + sh) + ccol])
                             for ccol in range(4))
                    d = max(d, dc)
                DEP[rg, half, sh] = d
    assert DEP.max() <= 128

    dep0 = DEP[0].reshape(16)
    order0 = np.argsort(-dep0, kind="stable")
    HA = int(dep0[order0[:GRP[0]]].max())
    HB1 = int(dep0[order0[GRP[0]:GRP[0] + GRP[1]]].max())
    HB2 = int(dep0[order0[GRP[0] + GRP[1]:]].max())
    pos0 = np.zeros(16, dtype=np.int64)
    for n, Sd in enumerate(order0):
        pos0[Sd] = n

    shapes = _shapes(HA, HB1, HB2)
    base, spec = [], []
    for c in range(NC):
        arrs = {k: np.zeros(v, dtype=_bf16) for k, v in shapes.items()}
        sp = {k: ([], [], [], []) for k in arrs}
        for rg in range(NI):
            for half in range(2):
                for sh in range(8):
                    name, mc, sc = _layout(rg, half, sh, pos0)
                    a = arrs[name]
                    cols = a.shape[1]
                    r0 = 0
                    for ccol in range(4):
                        i = int(imap[c, rg, sh % 4, ccol])
                        j = 4 * (8 * half + sh) + ccol
                        s = i * M + j
                        K = int(Ksq[i, j])
                        a[r0:r0 + K, mc:mc + 128] = \
                            Vt[s][0:K].astype(_bf16)
                        fi, si, ki, li = sp[name]
                        for k in range(K):
                            fb = (r0 + k) * cols + sc + 32 * ccol
                            fi.extend(range(fb, fb + 32))
                            si.extend([s] * 32)
                            ki.extend([k] * 32)
                            li.extend(range(32))
                        r0 += K
        base.append(arrs)
        spec.append({k: tuple(np.array(x, dtype=np.int64) for x in v)
                     for k, v in sp.items()})

    fac = dict(Ksq=Ksq, imap=imap, DEP=DEP, G01=G01, G2=G2,
               HA=HA, HB1=HB1, HB2=HB2, pos0=pos0, base=base, spec=spec)
    _cache["fac"] = fac
    return fac


def _build():
    if "nc" in _cache:
        return _cache["nc"]
    fac = _factors()
    DEP, pos0 = fac["DEP"], fac["pos0"]
    shapes = _shapes(fac["HA"], fac["HB1"], fac["HB2"])
    nc = bacc.Bacc(target_bir_lowering=False, debug=False)
    dram = {k: nc.declare_dram_parameter(k, list(v), BF16, isOutput=False)
            for k, v in shapes.items()}
    xa_d = nc.declare_dram_parameter("xa", [128, 2048], FP16, isOutput=False)
    out_d = nc.declare_dram_parameter("out", [1024, 512], FP16,
                                      isOutput=True)

    with tile.TileContext(nc) as tc:
        with (
            tc.tile_pool(name="tabs", bufs=1) as wpool,
            tc.tile_pool(name="cp", bufs=3) as cpool,
            tc.tile_pool(name="tx", bufs=3) as tpool,
            tc.tile_pool(name="t1", bufs=3) as rpool,
            tc.tile_pool(name="t2", bufs=3) as r2pool,
            tc.tile_pool(name="ps", bufs=2, space="PSUM") as psum,
        ):
            tabs = {k: wpool.tile(list(v), BF16, name=k)
                    for k, v in shapes.items()}
            xa = wpool.tile([128, 2048], FP16, name="xa")
            # single-ring (sync) input DMA in strict need order, sized
            # so each table lands just before its rg's matmuls.
            nc.sync.dma_start(tabs["l1a"][:], dram["l1a"][:])
            nc.sync.dma_start(tabs["l1b"][:], dram["l1b"][:])
            nc.sync.dma_start(tabs["l2a"][:], dram["l2a"][:])
            nc.sync.dma_start(xa[:], xa_d[:])
            nc.sync.dma_start(tabs["l2b"][:], dram["l2b"][:])
            nc.sync.dma_start(tabs["l3a"][:], dram["l3a"][:])
            nc.sync.dma_start(tabs["ha"][:], dram["ha"][:])
            nc.sync.dma_start(tabs["hb1"][:], dram["hb1"][:])
            nc.sync.dma_start(tabs["hb2"][:], dram["hb2"][:])
            nc.sync.dma_start(tabs["l3b"][:], dram["l3b"][:])

            for rg in RGORDER:
                ps = psum.tile([128, 2048], F32, tag="ps", name=f"ps{rg}")
                for half in range(2):
                    for sh in range(8):
                        name, mc, sc = _layout(rg, half, sh, pos0)
                        t = tabs[name]
                        K = int(DEP[rg, half, sh])
                        col = 128 * (8 * half + sh)
                        nc.tensor.matmul(
                            ps[:, col:col + 128],
                            t[0:K, sc:sc + 128],
                            t[0:K, mc:mc + 128],
                            start=True, stop=True)
                cp = cpool.tile([128, 2048], FP16, tag="cp", name=f"cp{rg}")
                tm = tpool.tile([128, 2048], FP16, tag="tx", name=f"tx{rg}")
                # per-half copy + multiply: copy-h0 starts after only
                # the first 8 matmuls; shortens both ends of the
                # pipeline at the same scalar/vector steady cost.
                for half in range(2):
                    lo, hi = 1024 * half, 1024 * (half + 1)
                    nc.scalar.copy(cp[:, lo:hi], ps[:, lo:hi])
                    nc.vector.tensor_tensor(tm[:, lo:hi], cp[:, lo:hi],
                                            xa[:, lo:hi],
                                            mybir.AluOpType.mult)
                t1 = rpool.tile([128, 1024], FP16, name=f"t1_{rg}",
                                tag="t1")
                nc.vector.tensor_tensor(t1[:], tm[:, 0:1024],
                                        tm[:, 1024:2048],
                                        mybir.AluOpType.add)
                t2 = r2pool.tile([128, 512], FP16, name=f"t2_{rg}",
                                 tag="t2")
                nc.vector.tensor_tensor(t2[:], t1[:, 0:512],
                                        t1[:, 512:1024],
                                        mybir.AluOpType.add)
                nc.sync.dma_start(out_d[128 * rg:128 * (rg + 1), :], t2[:])
    nc.compile()
    _cache["nc"] = nc
    return nc


def _pack_stat(P_):
    fac = _factors()
    G01, G2 = fac["G01"], fac["G2"]
    Pf = P_.reshape(M * M, O).astype(np.float32)
    statp = np.zeros((M * M, 32, 32), dtype=np.float32)
    statp[:128] = np.einsum('sg,sgkl->skl', Pf[:128], G01)
    statp[128:, 0:2] = np.einsum('sg,sgkl->skl', Pf[128:], G2)
    outs = []
    for c in range(NC):
        arrs = {k: v.copy() for k, v in fac["base"][c].items()}
        for name, (fi, si, ki, li) in fac["spec"][c].items():
            arrs[name].flat[fi] = statp[si, ki, li].astype(_bf16)
        outs.append(arrs)
    return outs


def _pack_x(x):
    # xa[32*ccol + l, 128*s + h] = x[32h+l, j], j = 4*s + ccol
    x4 = x.reshape(NH, NL, 16, 4)                 # [h, l, s, ccol]
    xa = np.ascontiguousarray(x4.transpose(3, 1, 2, 0)).reshape(128, 2048)
    return xa.astype(_fp16)


def _numpy_fallback(k, x, P_):
    out = np.zeros((B, M), dtype=np.float32)
    periods = (np.arange(M * M * O, dtype=np.float32) + 2.0).reshape(M, M, O)
    CH = 256
    for s0 in range(0, B, CH):
        kb = k[s0:s0 + CH].astype(np.float32)
        phi = np.cos(np.float32(TWO_PI) * kb[:, None, None, None]
                     / periods[None]).astype(np.float32)
        out[s0:s0 + CH] = np.einsum('bj,ijg,bijg->bi', x[s0:s0 + CH],
                                    P_.astype(np.float32), phi,
                                    optimize=True).astype(np.float32)
    return out


def kernel(k_tensor, token_indices, emb, P):
    global _last_results
    k = np.asarray(k_tensor, dtype=np.float32).reshape(B)
    tok = np.asarray(token_indices).astype(np.int64).reshape(B)
    emb_ = np.asarray(emb, dtype=np.float32)
    P_ = np.asarray(P, dtype=np.float32)
    x = emb_[tok]                                          # [B, 64]

    if not np.array_equal(k, np.arange(B, dtype=np.float32)):
        return _numpy_fallback(k, x, P_)

    fac = _factors()
    tabs = _pack_stat(P_)
    xa = _pack_x(x)
    nc = _build()
    in_maps = []
    for c in range(NC):
        m = {k2: np.ascontiguousarray(v) for k2, v in tabs[c].items()}
        m["xa"] = xa
        in_maps.append(m)
    res = run_bass_kernel_spmd(nc, in_maps, list(range(NC)))
    _last_results = res

    imap = fac["imap"]
    # out rows: 128*rg + 32*ccol + l; cols: 128*sg2 + h; b = 32h + l
    chains, idx = [], []
    for c in range(NC):
        od = res.results[c]["out"].astype(np.float32)      # [1024, 512]
        a = od.reshape(NI, 4, NL, 4, NH)                   # [rg,ccol,l,sg2,h]
        a = a.transpose(0, 3, 1, 4, 2)                     # [rg,sg2,ccol,h,l]
        chains.append(a.reshape(NI * 4 * 4, NH * NL))
        idx.append(imap[c].reshape(-1))
    chains = np.concatenate(chains, axis=0)                # [1024, 4096]
    idx = np.concatenate(idx)
    order = np.argsort(idx, kind="stable")
    grouped = chains[order].reshape(M, 16, B).sum(axis=1)  # [i, b]
    return np.ascontiguousarray(grouped.T)                 # [b, i]


# revision 15
# speedup vs baseline: 1.0430x; 1.0430x over previous
"""Trainium2 Bass kernel for nn_DualDescriptorTS.

Math:  Nk[b,i] = sum_{j,g} x[b,j] * P[i,j,g] * cos(2*pi*k[b]/p[i,j,g]),
       p[i,j,g] = i*1024 + j*16 + g + 2,  x = emb[token_indices].

Factorization (k = arange(B), b = 32*h + l): angle addition gives each
(i, j) slab D_{i,j}[l, h] = stat^T mov with mov P-independent and
numerically low rank (exactly 2 at a 1% tail for every i >= 2 slab;
only i in {0, 1} are heavier).  mov ~ A @ Q via SVD; P folds into the
tiny stationary factor statp = A^T stat per call.

v5 design (trace-driven):
  * Merged matmuls: the 4 ccol slabs of a (rg, half, sh) slot run as
    ONE matmul with a block-diagonal stationary [Ktot<=38, 128];
    heavy chains (i=0,1) spread one-per-core.  PE sustains ~107ns
    per LDWEIGHTS+MATMUL pair -> ~1.7us/rg.
  * One [128, 2048] PSUM tile per rg (4 banks, bufs=2): ONE scalar
    ACT copy downconverts the rg to fp16 (~1.97us), vector does ONE
    fp16 2x-mode multiply by x (~1.2us) and the whole fold
    (t1 [1024] + t2 [512], ~1.03us).  GpSimd is kept off the
    elementwise path entirely: measured SBUF contention bloats
    concurrent vector ops ~3x, costing more than gpsimd contributes.
  * Chains of 4 slabs {j, j+16, j+32, j+48}: the on-device fold
    cuts output to 128KB/rg.  The DMA fabric measures ~110GB/s
    aggregate with 2-4KB row-packets, so 2MB of output (chains-2)
    was the binding tail; 1MB clears it.  Host sums 16 partials/row.
  * Input DMA: one ring (sync), strict need order, sized so each
    table lands just before its row-group's matmuls; rg0 (heavy
    tables) processed late.  xa rides the same ring in two halves.
  * ~8us of framework preamble/epilogue (257 sem clears) is fixed.
"""
import numpy as np
import ml_dtypes

import concourse.bacc as bacc
import concourse.tile as tile
from concourse import mybir
from concourse.bass_utils import run_bass_kernel_spmd

F32 = mybir.dt.float32
BF16 = mybir.dt.bfloat16
FP16 = mybir.dt.float16
TWO_PI = 2.0 * np.pi

M, O, B = 64, 16, 4096
NC = 8            # cores
NI = 8            # row-groups (rgs) per core
NH, NL = 128, 32  # b = 32*h + l
TAU = 0.01        # relative Frobenius tail kept when truncating mov
GRP = [2, 4, 10]  # rg0 mms per depth-sorted rect (ha, hb1, hb2)

_bf16 = ml_dtypes.bfloat16
_fp16 = np.float16
_cache = {}
_last_results = None

RGORDER = [1, 2, 3, 4, 5, 0, 6, 7]


def _heavy_core(i, sg2, ccol):
    if i == 0:
        return ccol + 4 * (sg2 % 2)
    return ccol + 4 * ((sg2 + 1) % 2)


def _shapes(HA, HB1, HB2):
    return {"l1a": (8, 2048), "l1b": (8, 2048), "l2a": (8, 4096),
            "l2b": (8, 4096), "l3a": (8, 8192), "l3b": (8, 8192),
            "ha": (HA, GRP[0] * 256), "hb1": (HB1, GRP[1] * 256),
            "hb2": (HB2, GRP[2] * 256)}


def _layout(rg, half, sh, pos0):
    """(tensor_name, mov_col, stat_col) for mm (rg, half, sh)."""
    if rg == 0:
        n = int(pos0[8 * half + sh])
        if n < GRP[0]:
            return "ha", 256 * n, 256 * n + 128
        n -= GRP[0]
        if n < GRP[1]:
            return "hb1", 256 * n, 256 * n + 128
        n -= GRP[1]
        return "hb2", 256 * n, 256 * n + 128
    if rg == 1:
        name = "l1a" if half == 0 else "l1b"
        return name, 128 * sh, 1024 + 128 * sh
    if rg == 2:
        name, off = "l2a", 0
    elif rg == 3:
        name, off = "l2b", 0
    elif rg <= 5:
        name, off = "l3a", 4096 * (rg - 4)
    else:
        name, off = "l3b", 4096 * (rg - 6)
    mc = off + 2048 * half + 128 * sh
    return name, mc, mc + 1024


def _factors():
    if "fac" in _cache:
        return _cache["fac"]
    h = np.arange(NH, dtype=np.float64)
    l = np.arange(NL, dtype=np.float64)
    ig = np.arange(M, dtype=np.float64)[:, None, None]
    jg = np.arange(M, dtype=np.float64)[None, :, None]
    gg = np.arange(O, dtype=np.float64)[None, None, :]
    theta = TWO_PI / (1024.0 * ig + 16.0 * jg + gg + 2.0)  # [i, j, g]
    a1 = theta[..., None] * (32.0 * h)
    mov = np.concatenate([np.cos(a1), np.sin(a1)], axis=2).reshape(
        M * M, 32, NH)
    U, S, Vt = np.linalg.svd(mov, full_matrices=False)
    fro = np.sqrt((S ** 2).sum(1))
    tail = np.sqrt(np.cumsum((S ** 2)[:, ::-1], axis=1))[:, ::-1] / fro[:, None]
    Ks = np.maximum(
        np.array([np.searchsorted(-tail[s], -TAU) for s in range(M * M)]), 1)
    Ksq = Ks.reshape(M, M)
    A = (U * S[:, None, :])                                 # [4096, 32, 32]

    thl = theta.reshape(M * M, O)[:, :, None] * l           # [s, g, l]
    cl, sl = np.cos(thl), np.sin(thl)
    G01 = (A[:128, :O, :, None] * cl[:128, :, None, :]
           - A[:128, O:, :, None] * sl[:128, :, None, :]).astype(np.float32)
    G2 = (A[128:, :O, :2, None] * cl[128:, :, None, :]
          - A[128:, O:, :2, None] * sl[128:, :, None, :]).astype(np.float32)

    # chain map: imap[c, rg, sg2, ccol] = i; chain covers the 4 slabs
    # j = 4*(8*half + sg2 + 4*m') + ccol, i.e. {j0, j0+16, j0+32, j0+48}
    imap = np.zeros((NC, NI, 4, 4), dtype=np.int64)
    for sg2 in range(4):
        for ccol in range(4):
            imap[_heavy_core(0, sg2, ccol), 0, sg2, ccol] = 0
            imap[_heavy_core(1, sg2, ccol), 0, sg2, ccol] = 1
            light = []
            for rg in range(NI):
                for c in range(NC):
                    if rg == 0 and (c == _heavy_core(0, sg2, ccol)
                                    or c == _heavy_core(1, sg2, ccol)):
                        continue
                    light.append((c, rg))
            assert len(light) == 62
            for n, (c, rg) in enumerate(light):
                imap[c, rg, sg2, ccol] = n + 2

    DEP = np.zeros((NI, 2, 8), dtype=np.int64)
    for rg in range(NI):
        for half in range(2):
            for sh in range(8):
                d = 0
                for c in range(NC):
                    dc = sum(int(Ksq[imap[c, rg, sh % 4, ccol],
                                     4 * (8 * half + sh) + ccol])
                             for ccol in range(4))
                    d = max(d, dc)
                DEP[rg, half, sh] = d
    assert DEP.max() <= 128

    dep0 = DEP[0].reshape(16)
    order0 = np.argsort(-dep0, kind="stable")
    HA = int(dep0[order0[:GRP[0]]].max())
    HB1 = int(dep0[order0[GRP[0]:GRP[0] + GRP[1]]].max())
    HB2 = int(dep0[order0[GRP[0] + GRP[1]:]].max())
    pos0 = np.zeros(16, dtype=np.int64)
    for n, Sd in enumerate(order0):
        pos0[Sd] = n

    shapes = _shapes(HA, HB1, HB2)
    base, spec = [], []
    for c in range(NC):
        arrs = {k: np.zeros(v, dtype=_bf16) for k, v in shapes.items()}
        sp = {k: ([], [], [], []) for k in arrs}
        for rg in range(NI):
            for half in range(2):
                for sh in range(8):
                    name, mc, sc = _layout(rg, half, sh, pos0)
                    a = arrs[name]
                    cols = a.shape[1]
                    r0 = 0
                    for ccol in range(4):
                        i = int(imap[c, rg, sh % 4, ccol])
                        j = 4 * (8 * half + sh) + ccol
                        s = i * M + j
                        K = int(Ksq[i, j])
                        a[r0:r0 + K, mc:mc + 128] = \
                            Vt[s][0:K].astype(_bf16)
                        fi, si, ki, li = sp[name]
                        for k in range(K):
                            fb = (r0 + k) * cols + sc + 32 * ccol
                            fi.extend(range(fb, fb + 32))
                            si.extend([s] * 32)
                            ki.extend([k] * 32)
                            li.extend(range(32))
                        r0 += K
        base.append(arrs)
        spec.append({k: tuple(np.array(x, dtype=np.int64) for x in v)
                     for k, v in sp.items()})

    fac = dict(Ksq=Ksq, imap=imap, DEP=DEP, G01=G01, G2=G2,
               HA=HA, HB1=HB1, HB2=HB2, pos0=pos0, base=base, spec=spec)
    _cache["fac"] = fac
    return fac


def _build():
    if "nc" in _cache:
        return _cache["nc"]
    fac = _factors()
    DEP, pos0 = fac["DEP"], fac["pos0"]
    shapes = _shapes(fac["HA"], fac["HB1"], fac["HB2"])
    nc = bacc.Bacc(target_bir_lowering=False, debug=False)
    dram = {k: nc.declare_dram_parameter(k, list(v), BF16, isOutput=False)
            for k, v in shapes.items()}
    xa_d = nc.declare_dram_parameter("xa", [128, 2048], FP16, isOutput=False)
    out_d = nc.declare_dram_parameter("out", [1024, 512], FP16,
                                      isOutput=True)

    with tile.TileContext(nc) as tc:
        with (
            tc.tile_pool(name="tabs", bufs=1) as wpool,
            tc.tile_pool(name="cp", bufs=3) as cpool,
            tc.tile_pool(name="tx", bufs=3) as tpool,
            tc.tile_pool(name="t1", bufs=3) as rpool,
            tc.tile_pool(name="t2", bufs=3) as r2pool,
            tc.tile_pool(name="ps", bufs=2, space="PSUM") as psum,
        ):
            tabs = {k: wpool.tile(list(v), BF16, name=k)
                    for k, v in shapes.items()}
            xa = wpool.tile([128, 2048], FP16, name="xa")
            # single-ring (sync) input DMA in strict need order, sized
            # so each table lands just before its rg's matmuls.
            nc.sync.dma_start(tabs["l1a"][:], dram["l1a"][:])
            nc.sync.dma_start(tabs["l1b"][:], dram["l1b"][:])
            nc.sync.dma_start(tabs["l2a"][:], dram["l2a"][:])
            nc.sync.dma_start(xa[:, 0:1024], xa_d[:, 0:1024])
            nc.sync.dma_start(xa[:, 1024:2048], xa_d[:, 1024:2048])
            nc.sync.dma_start(tabs["l2b"][:], dram["l2b"][:])
            nc.sync.dma_start(tabs["l3a"][:], dram["l3a"][:])
            nc.sync.dma_start(tabs["ha"][:], dram["ha"][:])
            nc.sync.dma_start(tabs["hb1"][:], dram["hb1"][:])
            nc.sync.dma_start(tabs["hb2"][:], dram["hb2"][:])
            nc.sync.dma_start(tabs["l3b"][:], dram["l3b"][:])

            for rg in RGORDER:
                ps = psum.tile([128, 2048], F32, tag="ps", name=f"ps{rg}")
                for half in range(2):
                    for sh in range(8):
                        name, mc, sc = _layout(rg, half, sh, pos0)
                        t = tabs[name]
                        K = int(DEP[rg, half, sh])
                        col = 128 * (8 * half + sh)
                        nc.tensor.matmul(
                            ps[:, col:col + 128],
                            t[0:K, sc:sc + 128],
                            t[0:K, mc:mc + 128],
                            start=True, stop=True)
                cp = cpool.tile([128, 2048], FP16, tag="cp", name=f"cp{rg}")
                tm = tpool.tile([128, 2048], FP16, tag="tx", name=f"tx{rg}")
                if rg in (RGORDER[0], RGORDER[-1]):
                    # edge rgs: per-half copy + multiply shortens the
                    # pipeline fill (copy-h0 needs only 8 matmuls) and
                    # the drain (multiply starts before copy-h1 ends).
                    for half in range(2):
                        lo, hi = 1024 * half, 1024 * (half + 1)
                        nc.scalar.copy(cp[:, lo:hi], ps[:, lo:hi])
                        nc.vector.tensor_tensor(tm[:, lo:hi],
                                                cp[:, lo:hi],
                                                xa[:, lo:hi],
                                                mybir.AluOpType.mult)
                else:
                    # steady rgs: whole-tile ops amortize fixed
                    # per-op overhead (scalar 1.97us, vector 1.22us)
                    nc.scalar.copy(cp[:], ps[:])
                    nc.vector.tensor_tensor(tm[:], cp[:], xa[:],
                                            mybir.AluOpType.mult)
                t1 = rpool.tile([128, 1024], FP16, name=f"t1_{rg}",
                                tag="t1")
                nc.vector.tensor_tensor(t1[:], tm[:, 0:1024],
                                        tm[:, 1024:2048],
                                        mybir.AluOpType.add)
                t2 = r2pool.tile([128, 512], FP16, name=f"t2_{rg}",
                                 tag="t2")
                nc.vector.tensor_tensor(t2[:], t1[:, 0:512],
                                        t1[:, 512:1024],
                                        mybir.AluOpType.add)
                nc.sync.dma_start(out_d[128 * rg:128 * (rg + 1), :], t2[:])
    nc.compile()
    _cache["nc"] = nc
    return nc


def _pack_stat(P_):
    fac = _factors()
    G01, G2 = fac["G01"], fac["G2"]
    Pf = P_.reshape(M * M, O).astype(np.float32)
    statp = np.zeros((M * M, 32, 32), dtype=np.float32)
    statp[:128] = np.einsum('sg,sgkl->skl', Pf[:128], G01)
    statp[128:, 0:2] = np.einsum('sg,sgkl->skl', Pf[128:], G2)
    outs = []
    for c in range(NC):
        arrs = {k: v.copy() for k, v in fac["base"][c].items()}
        for name, (fi, si, ki, li) in fac["spec"][c].items():
            arrs[name].flat[fi] = statp[si, ki, li].astype(_bf16)
        outs.append(arrs)
    return outs


def _pack_x(x):
    # xa[32*ccol + l, 128*s + h] = x[32h+l, j], j = 4*s + ccol
    x4 = x.reshape(NH, NL, 16, 4)                 # [h, l, s, ccol]
    xa = np.ascontiguousarray(x4.transpose(3, 1, 2, 0)).reshape(128, 2048)
    return xa.astype(_fp16)


def _numpy_fallback(k, x, P_):
    out = np.zeros((B, M), dtype=np.float32)
    periods = (np.arange(M * M * O, dtype=np.float32) + 2.0).reshape(M, M, O)
    CH = 256
    for s0 in range(0, B, CH):
        kb = k[s0:s0 + CH].astype(np.float32)
        phi = np.cos(np.float32(TWO_PI) * kb[:, None, None, None]
                     / periods[None]).astype(np.float32)
        out[s0:s0 + CH] = np.einsum('bj,ijg,bijg->bi', x[s0:s0 + CH],
                                    P_.astype(np.float32), phi,
                                    optimize=True).astype(np.float32)
    return out


def kernel(k_tensor, token_indices, emb, P):
    global _last_results
    k = np.asarray(k_tensor, dtype=np.float32).reshape(B)
    tok = np.asarray(token_indices).astype(np.int64).reshape(B)
    emb_ = np.asarray(emb, dtype=np.float32)
    P_ = np.asarray(P, dtype=np.float32)
    x = emb_[tok]                                          # [B, 64]

    if not np.array_equal(k, np.arange(B, dtype=np.float32)):
        return _numpy_fallback(k, x, P_)

    fac = _factors()
    tabs = _pack_stat(P_)
    xa = _pack_x(x)
    nc = _build()
    in_maps = []
    for c in range(NC):
        m = {k2: np.ascontiguousarray(v) for k2, v in tabs[c].items()}
        m["xa"] = xa
        in_maps.append(m)
    res = run_bass_kernel_spmd(nc, in_maps, list(range(NC)))
    _last_results = res

    imap = fac["imap"]
    # out rows: 128*rg + 32*ccol + l; cols: 128*sg2 + h; b = 32h + l
    chains, idx = [], []
    for c in range(NC):
        od = res.results[c]["out"].astype(np.float32)      # [1024, 512]
        a = od.reshape(NI, 4, NL, 4, NH)                   # [rg,ccol,l,sg2,h]
        a = a.transpose(0, 3, 1, 4, 2)                     # [rg,sg2,ccol,h,l]
        chains.append(a.reshape(NI * 4 * 4, NH * NL))
        idx.append(imap[c].reshape(-1))
    chains = np.concatenate(chains, axis=0)                # [1024, 4096]
    idx = np.concatenate(idx)
    order = np.argsort(idx, kind="stable")
    grouped = chains[order].reshape(M, 16, B).sum(axis=1)  # [i, b]
    return np.ascontiguousarray(grouped.T)                 # [b, i]
